# revision 1
# baseline (speedup 1.0000x reference)
"""DIEN (Deep Interest Evolution Network) Bass/Tile kernel for Trainium2.

Strategy: pure data parallel over batch. Each of the 8 NeuronCores gets
B_local = 128 batch rows; embedding tables and all weights are replicated.

On-chip layout is feature-major: [feature -> partitions, batch -> free].
Embedding tables are held in bf16 (padded so rows are 128-wide), gathered
rows go through the DMA xbar transpose into feature-major bf16 tiles, and
the GRU input projections (gx) are computed in batched bf16 matmuls
directly into the same PSUM tiles that the sequential scan then
accumulates gh = Whh @ h onto (matmul start=False), so no elementwise add
is needed for gx + gh. Sequence masking is folded into the z gate by
adding 40*(1-mask) to the z pre-activation (sigmoid saturates to exactly
1.0f, freezing h), and interests are masked in one batched multiply per
chunk. Biases are exact and free: GRU input-side biases ride on an
appended ones-column of the category table; per-feature biases use the
scalar/bias slots of activation / scalar_tensor_tensor ops. The
recurrent state math stays in fp32.
"""

import os
import sys

for _p in ("/opt/trn_rl_repo", "/root/.axon_site/_ro/trn_rl_repo"):
    if os.path.isdir(_p) and _p not in sys.path:
        sys.path.insert(0, _p)

import numpy as np
from ml_dtypes import bfloat16 as np_bf16

import concourse.bacc as bacc
import concourse.bass as bass
import concourse.mybir as mybir
import concourse.tile as tile
from concourse.bass import IndirectOffsetOnAxis
from concourse.bass_utils import run_bass_kernel_spmd

F32 = mybir.dt.float32
BF16 = mybir.dt.bfloat16
I32 = mybir.dt.int32
AF = mybir.ActivationFunctionType
OP = mybir.AluOpType

B, S, D, H, F = 1024, 200, 128, 128, 10
NU, NI, NC = 100000, 100000, 1000
DC = D // 2          # 64
DH = D + DC          # 192
NCORES = 8
BL = B // NCORES     # 128 batch rows per core
TC = 4               # timesteps per pipeline chunk
KCH_HOST = [D, D, DC, H, H, F]

# packed-weight layouts: (name, rows, cols) -> column ranges of one blob,
# loaded with a single DMA (HWDGE has ~625ns fixed cost per descriptor)
WBF = [
    ("wih_i", D, 3 * H), ("wih_c", 128, 3 * H), ("whh", H, 3 * H),
    ("w1i", H, 80), ("w1ti", D, 80), ("w1tc", DC, 80),
    ("w2", 80, 40), ("w3r", 40, 128),
    ("wir", H, H), ("wiha", H, H), ("whr", H, H), ("whha", H, H),
    ("fc1k0", D, 256), ("fc1k1", D, 256), ("fc1k2", DC, 256),
    ("fc1k3", H, 256), ("fc1k4", H, 256), ("fc1k5", F, 256),
    ("fc2a", 128, 128), ("fc2b", 128, 128), ("fc3", 128, 64), ("fc4", 64, 1),
    ("onesb", 128, B // 8), ("featT", F, B // 8), ("id80", 80, 80),
]
WF32 = [("bhhn", H, 1), ("b1", 80, 1), ("b2", 40, 1), ("b3r", 128, 1),
        ("br", H, 1), ("bh", H, 1), ("fb1a", 128, 1), ("fb1b", 128, 1),
        ("fb2", 128, 1), ("fb3", 64, 1), ("fb4", 1, 1)]


def _offsets(layout):
    offs, o = {}, 0
    for name, rows, cols in layout:
        offs[name] = (o, rows, cols)
        o += cols
    return offs, o


WBF_OFF, WBF_N = _offsets(WBF)
WF32_OFF, WF32_N = _offsets(WF32)

_BUILT = {}


def _ap3(base, off, mid_step, mid_n, inner_n):
    """3D view [P, mid_n, inner_n] of a 2D tile AP at free-offset off."""
    a = base[:, off:off + 1]
    return bass.AP(a.tensor, a.offset, [a.ap[0], [mid_step, mid_n], [1, inner_n]])


def _build(ss):
    """Build + compile the single-core module for ss timesteps."""
    nc = bacc.Bacc("TRN2", target_bir_lowering=False, debug=False)
    nch = ss // TC

    def din(name, shape, dt=F32):
        return nc.dram_tensor(name, list(shape), dt, kind="ExternalInput").ap()

    # per-core data
    ids3 = din("ids3", [BL, 3], I32)        # [uid, aid, NI+cid]
    hidx = din("hidx", [BL, 2 * ss], I32)   # [item_t, NI+cat_t]; pad steps
    # redirect to rows NI+NC (zero item) / NI+NC+1 (pad-cat: +40 z bias,
    # mask flag 0), so masking rides on the gather + gx matmul for free.
    # tables (replicated, bf16): utab rows; tabic = [item_table ; cat_table
    # padded to 128 with mask flag @64, ones @65, z-pad flag @66; 2 pad rows]
    utab = din("utab", [NU, D], BF16)
    tabic = din("tabic", [NI + NC + 2, 128], BF16)
    # all small weights ride in two packed blobs (one DMA each)
    wpk = din("wpk", [128, WBF_N], BF16)
    fpk = din("fpk", [128, WF32_N])

    out = nc.dram_tensor("out", [1, BL], F32, kind="ExternalOutput").ap()

    with tile.TileContext(nc) as tc:
        with tc.tile_pool(name="persist", bufs=1) as pp:
            # ---- persistent SBUF tensors ----
            wpk_s = pp.tile([128, WBF_N], BF16, name="wpk_s")
            nc.sync.dma_start(out=wpk_s[:], in_=wpk[:])
            fpk_s = pp.tile([128, WF32_N], F32, name="fpk_s")
            nc.sync.dma_start(out=fpk_s[:], in_=fpk[:])

            def wv(name):
                o, r, cs = WBF_OFF[name]
                return wpk_s[:r, o:o + cs]

            def fv(name):
                o, r, _ = WF32_OFF[name]
                return fpk_s[:r, o:o + 1]

            wih_i_s = wv("wih_i")
            wih_c_s = wv("wih_c")
            whh_s = wv("whh")
            bhhn_s = fv("bhhn")
            w1i_s = wv("w1i")
            w1ti_s = wv("w1ti")
            w1tc_s = wv("w1tc")
            b1_s = fv("b1")
            w2_s = wv("w2")
            b2_s = fv("b2")
            w3r_s = wv("w3r")
            b3r_s = fv("b3r")
            wir_s = wv("wir")
            wiha_s = wv("wiha")
            whr_s = wv("whr")
            whha_s = wv("whha")
            br_s = fv("br")
            bh_s = fv("bh")
            fc1k_s = [wv(f"fc1k{j}") for j in range(6)]
            fb1a_s = fv("fb1a")
            fb1b_s = fv("fb1b")
            fb2_s = fv("fb2")
            fc2a_s = wv("fc2a")
            fc2b_s = wv("fc2b")
            fc3_s = wv("fc3")
            fb3_s = fv("fb3")
            fc4_s = wv("fc4")
            fb4_s = fv("fb4")
            ob_o = WBF_OFF["onesb"][0]
            ones64_s = wpk_s[DC:DC + 1, ob_o:ob_o + BL]   # row 64 of onesb == 1
            featT_s = wv("featT")
            id80_s = wv("id80")

            h0 = pp.tile([128, BL], BF16, name="h0")
            nc.gpsimd.memset(h0[:], 0)

            userT = pp.tile([D, BL], BF16, name="userT")
            itemT = pp.tile([D, BL], BF16, name="itemT")
            catT = pp.tile([128, BL], BF16, name="catT")
            attc = pp.tile([80, BL], BF16, name="attc")  # W1t @ tgt + b1
            ibig = pp.tile([128, ss * BL], BF16, name="ibig")  # masked interests
            hau = pp.tile([128, BL], BF16, name="hau")  # AUGRU state (final)

            # ---- phase 1: target/user embedding lookups ----
            with tc.tile_pool(name="p1s", bufs=1) as p1:
                g_u = p1.tile([BL, D], BF16, name="g_u")
                g_a = p1.tile([BL, D], BF16, name="g_a")
                g_c = p1.tile([BL, 128], BF16, name="g_c")
                ids_s = p1.tile([BL, 3], I32, name="ids_s")
                nc.sync.dma_start(out=ids_s[:], in_=ids3[:])
                for j, (g, table) in enumerate(((g_u, utab), (g_a, tabic), (g_c, tabic))):
                    nc.gpsimd.indirect_dma_start(
                        out=g[:], out_offset=None, in_=table[:],
                        in_offset=IndirectOffsetOnAxis(ap=ids_s[:, j:j + 1], axis=0),
                    )
                for g, dst in ((g_u, userT), (g_a, itemT), (g_c, catT)):
                    nc.sync.dma_start(out=dst[:], in_=g[:], transpose=True)
                # attention constant: W1t @ [itemT; catT] + b1
                with tc.tile_pool(name="p1p", bufs=1, space="PSUM") as q1:
                    psc = q1.tile([80, BL], F32, name="psc")
                    nc.tensor.matmul(out=psc[:], lhsT=w1ti_s[:], rhs=itemT[:], start=True, stop=False)
                    nc.tensor.matmul(out=psc[:], lhsT=w1tc_s[:], rhs=catT[:DC, :], start=False, stop=True)
                    nc.scalar.activation(out=attc[:], in_=psc[:], func=AF.Identity, bias=b1_s[:, :1])

            # ---- phases 2+3 merged, software-pipelined ----
            # iter c: DMA-prefetch chunk c+1, GRU scan chunk c, attention
            # for chunk c-1 spread over step slots, AUGRU scan chunk c-2.
            NTB = TC * BL
            with (
                tc.tile_pool(name="p2s", bufs=1) as p2,
                tc.tile_pool(name="p2p", bufs=1, space="PSUM") as q2,
            ):
                h_prev = h0[:]
                ha_prev = h0[:]
                attc_bc = bass.AP(attc[:].tensor, attc[:].offset,
                                  [attc[:].ap[0], [0, TC], [1, BL]])
                fetched = {}
                a3s_by = {}

                def fetch_dma(c):
                    """idx DMA (SP) and the combined gather (Pool)."""
                    t0 = c * TC
                    idx = p2.tile([BL, 2 * TC], I32, name="idx", tag="idx", bufs=2)
                    nc.sync.dma_start(out=idx[:], in_=hidx[:, 2 * t0:2 * (t0 + TC)])
                    gic = p2.tile([BL, 2 * TC * 128], BF16, name="gic", tag="gic", bufs=2)
                    nc.gpsimd.indirect_dma_start(
                        out=gic[:], out_offset=None, in_=tabic[:],
                        in_offset=IndirectOffsetOnAxis(ap=idx[:, :2 * TC], axis=0))
                    fetched[c] = [gic, None, None]

                def fetch_tr(c):
                    """xbar transposes (SP/HWDGE), issued once the gather landed."""
                    gic = fetched[c][0]
                    hti = p2.tile([D, NTB], BF16, name="hti", tag="hti", bufs=2)
                    htc = p2.tile([128, NTB], BF16, name="htc", tag="htc", bufs=2)
                    for t in range(TC):
                        nc.sync.dma_start(out=hti[:, t * BL:(t + 1) * BL],
                                          in_=gic[:, (2 * t) * 128:(2 * t + 1) * 128],
                                          transpose=True)
                        nc.sync.dma_start(out=htc[:, t * BL:(t + 1) * BL],
                                          in_=gic[:, (2 * t + 1) * 128:(2 * t + 2) * 128],
                                          transpose=True)
                    fetched[c][1] = hti
                    fetched[c][2] = htc

                pg_by = {}
                proj_by = {}

                def emit_gx(c):
                    """gx matmuls for chunk c (emitted during iter c-1 slot 3)."""
                    _, hti, htc = fetched.pop(c)
                    pg_rz = q2.tile([128, 2 * NTB], F32, name="pg_rz", tag="pg_rz", bufs=1)
                    pg_n = q2.tile([128, NTB], F32, name="pg_n", tag="pg_n", bufs=1)
                    for g, (pg, o) in enumerate(((pg_rz, 0), (pg_rz, NTB), (pg_n, 0))):
                        dst = pg[:, o:o + NTB]
                        nc.tensor.matmul(out=dst, skip_group_check=True,
                                         lhsT=wih_i_s[:, g * H:(g + 1) * H],
                                         rhs=hti[:], start=True, stop=False)
                        # pg_n has no per-step accumulation: close its group
                        nc.tensor.matmul(out=dst, skip_group_check=True,
                                         lhsT=wih_c_s[:, g * H:(g + 1) * H],
                                         rhs=htc[:], start=False, stop=(g == 2))
                    pg_by[c] = (pg_rz, pg_n, htc)

                def emit_proj(cb):
                    """AUGRU input projections for chunk cb (slot 3 of iter cb+1)."""
                    isl = ibig[:, cb * NTB:(cb + 1) * NTB]
                    pgr = q2.tile([128, NTB], F32, name="pgr", tag="pgr", bufs=1)
                    pgn = q2.tile([128, NTB], F32, name="pgn", tag="pgn", bufs=1)
                    nc.tensor.matmul(out=pgr[:], skip_group_check=True,
                                     lhsT=wir_s[:], rhs=isl, start=True, stop=False)
                    nc.tensor.matmul(out=pgn[:], skip_group_check=True,
                                     lhsT=wiha_s[:], rhs=isl, start=True, stop=False)
                    proj_by[cb] = (pgr, pgn)

                for c in range(nch + 2):
                    if c == 0:
                        fetch_dma(0)
                        fetch_tr(0)
                    if c + 1 < nch:
                        fetch_dma(c + 1)

                    if c < nch:
                        t0 = c * TC
                        emit_gx(c)
                        pg_rz, pg_n, htc_c = pg_by.pop(c)
                        # partition-replicated mask for interests (flag row 64)
                        mrep = q2.tile([128, NTB], F32, name="mrep", tag="mrep", bufs=1)
                        nc.tensor.matmul(out=mrep[:], lhsT=ones64_s,
                                         rhs=htc_c[DC:DC + 1, :], start=True, stop=True)
                        hc = p2.tile([128, NTB], BF16, name="hc", tag="hc", bufs=2)

                    att = None
                    if 1 <= c <= nch:
                        # -- attention chunk c-1, spread over step slots --
                        ca = c - 1
                        isl_att = ibig[:, ca * NTB:(ca + 1) * NTB]
                        pa1 = q2.tile([128, NTB], F32, name="pa1", tag="patt", bufs=1)
                        a1s = p2.tile([80, NTB], BF16, name="a1s", tag="a1s", bufs=2)
                        a2s = p2.tile([40, NTB], BF16, name="a2s", tag="a2s", bufs=2)
                        a3s_new = p2.tile([128, NTB], BF16, name="a3s", tag="a3s", bufs=2)
                        a3s_by[ca] = a3s_new

                        def att0(pa1=pa1, isl_att=isl_att):
                            nc.tensor.matmul(out=pa1[:80, :], lhsT=w1i_s[:], rhs=isl_att,
                                             start=True, stop=False)
                            nc.tensor.matmul(out=pa1[:80, :], lhsT=id80_s[:], rhs=attc_bc,
                                             start=False, stop=True)

                        def att1(pa1=pa1, a1s=a1s):
                            nc.scalar.activation(out=a1s[:], in_=pa1[:80, :], func=AF.Relu)
                            pa2 = q2.tile([128, NTB], F32, name="pa2", tag="patt", bufs=1)
                            nc.tensor.matmul(out=pa2[:40, :], lhsT=w2_s[:], rhs=a1s[:],
                                             start=True, stop=True)
                            att.append(pa2)

                        def att2(a2s=a2s):
                            pa2 = att.pop()
                            nc.scalar.activation(out=a2s[:], in_=pa2[:40, :], func=AF.Relu,
                                                 bias=b2_s[:, :1])
                            pa3 = q2.tile([128, NTB], F32, name="pa3", tag="patt", bufs=1)
                            nc.tensor.matmul(out=pa3[:], lhsT=w3r_s[:], rhs=a2s[:],
                                             start=True, stop=True)
                            att.append(pa3)

                        def att3(a2s=a2s, a3s_new=a3s_new):
                            pa3 = att.pop()
                            nc.scalar.activation(out=a3s_new[:], in_=pa3[:], func=AF.Sigmoid,
                                                 bias=b3r_s[:, :1])

                        att = []
                        att_pieces = [att0, att1, att2, att3]

                    if c >= 2:
                        # -- AUGRU chunk c-2 (projections emitted last iter) --
                        cb = c - 2
                        a3s = a3s_by.pop(cb)
                        emit_proj(cb)
                        pgr, pgn = proj_by.pop(cb)

                    for t in range(TC):
                        sl = slice(t * BL, (t + 1) * BL)
                        last = t == TC - 1
                        do_g = c < nch
                        do_a = c >= 2
                        # --- PE: GRU gate MMs, AUGRU r MM ---
                        if do_g:
                            slz = slice(NTB + t * BL, NTB + (t + 1) * BL)
                            nc.tensor.matmul(out=pg_rz[:, sl], skip_group_check=True,
                                             lhsT=whh_s[:, 0:H], rhs=h_prev,
                                             start=False, stop=last)
                            nc.tensor.matmul(out=pg_rz[:, slz], skip_group_check=True,
                                             lhsT=whh_s[:, H:2 * H], rhs=h_prev,
                                             start=False, stop=last)
                            ngh = q2.tile([128, BL], F32, name="ngh", tag="ngh", bufs=1)
                            nc.tensor.matmul(out=ngh[:], lhsT=whh_s[:, 2 * H:3 * H],
                                             rhs=h_prev, start=True, stop=True)
                        if do_a:
                            nc.tensor.matmul(out=pgr[:, sl], skip_group_check=True,
                                             lhsT=whr_s[:], rhs=ha_prev,
                                             start=False, stop=last)
                        # --- Act: GRU sigmoid, AUGRU sigmoid ---
                        if do_g:
                            rs = p2.tile([128, 2 * BL], F32, name="rs", tag="rs", bufs=2)
                            nc.scalar.activation(out=_ap3(rs, 0, BL, 2, BL),
                                                 in_=_ap3(pg_rz, t * BL, NTB, 2, BL),
                                                 func=AF.Sigmoid)
                        if do_a:
                            ss_ = p2.tile([128, BL], F32, name="ss_", tag="ss_", bufs=2)
                            nc.scalar.activation(out=ss_[:], in_=pgr[:, sl], func=AF.Sigmoid,
                                                 bias=br_s[:, :1])
                        # --- attention piece for chunk c-1 (engine idle windows) ---
                        if att is not None:
                            att_pieces[t]()
                        # --- DVE: GRU t1,t2 (tanh feed); AUGRU uu/hu ---
                        if do_g:
                            t1 = p2.tile([128, BL], F32, name="t1", tag="t1", bufs=2)
                            nc.vector.scalar_tensor_tensor(
                                out=t1[:], in0=ngh[:], scalar=bhhn_s[:, :1], in1=rs[:, 0:BL],
                                op0=OP.add, op1=OP.mult)
                            # t2 = gxn + r*(ghn+bhhn): DVE reads pg_n psum directly
                            t2 = p2.tile([128, BL], F32, name="t2", tag="t2", bufs=2)
                            nc.vector.tensor_tensor(out=t2[:], in0=t1[:], in1=pg_n[:, sl],
                                                    op=OP.add)
                        if do_a:
                            uu = p2.tile([128, BL], F32, name="uu", tag="uu", bufs=2)
                            nc.vector.tensor_tensor(out=uu[:], in0=a3s[:, sl], in1=ss_[:],
                                                    op=OP.mult)
                            hu = p2.tile([128, BL], BF16, name="hu", tag="hu", bufs=2)
                            nc.vector.tensor_tensor(out=hu[:], in0=ha_prev, in1=uu[:],
                                                    op=OP.mult)
                            nc.tensor.matmul(out=pgn[:, sl], skip_group_check=True,
                                             lhsT=whha_s[:], rhs=hu[:],
                                             start=False, stop=last)
                        # --- DVE: off-chain elementwise (zbar, w) ---
                        if do_g:
                            zbar = p2.tile([128, BL], F32, name="zbar", tag="zbar", bufs=2)
                            nc.vector.tensor_scalar(out=zbar[:], in0=rs[:, BL:2 * BL],
                                                    scalar1=-1.0, scalar2=1.0,
                                                    op0=OP.mult, op1=OP.add)
                            w_ = p2.tile([128, BL], F32, name="w_", tag="w_", bufs=2)
                            nc.vector.tensor_tensor(out=w_[:], in0=rs[:, BL:2 * BL],
                                                    in1=h_prev, op=OP.mult)
                        # --- Act: GRU tanh, AUGRU tanh; DVE tails ---
                        if do_g:
                            nn = p2.tile([128, BL], F32, name="nn", tag="nn", bufs=2)
                            nc.scalar.activation(out=nn[:], in_=t2[:], func=AF.Tanh)
                        if do_a:
                            ht_ = p2.tile([128, BL], F32, name="ht_", tag="ht_", bufs=2)
                            nc.scalar.activation(out=ht_[:], in_=pgn[:, sl], func=AF.Tanh,
                                                 bias=bh_s[:, :1])
                        if do_g:
                            # h' = (1-z)*n + z*h
                            u_ = p2.tile([128, BL], F32, name="u_", tag="u_", bufs=2)
                            nc.vector.tensor_tensor(out=u_[:], in0=zbar[:], in1=nn[:],
                                                    op=OP.mult)
                            nc.vector.tensor_tensor(out=hc[:, sl], in0=u_[:], in1=w_[:],
                                                    op=OP.add)
                            h_prev = hc[:, sl]
                        if do_a:
                            s2 = p2.tile([128, BL], F32, name="s2", tag="s2", bufs=2)
                            # s2 = h - h*u (runs in the Atanh wait window)
                            nc.vector.scalar_tensor_tensor(
                                out=s2[:], in0=hu[:], scalar=-1.0, in1=ha_prev,
                                op0=OP.mult, op1=OP.add)
                            qq = p2.tile([128, BL], F32, name="qq", tag="qq", bufs=2)
                            nc.vector.tensor_tensor(out=qq[:], in0=uu[:], in1=ht_[:],
                                                    op=OP.mult)
                            is_last = (c == nch + 1 and last)
                            dsth = hau if is_last else p2.tile([128, BL], BF16,
                                                               name="han", tag="han", bufs=2)
                            nc.vector.tensor_tensor(out=dsth[:], in0=s2[:], in1=qq[:],
                                                    op=OP.add)
                            ha_prev = dsth[:]
                        # transposes for chunk c+1 once its gather landed
                        if t == 1 and c + 1 in fetched:
                            fetch_tr(c + 1)

                    if c < nch:
                        # masked interests for this chunk (bf16)
                        nc.vector.tensor_tensor(out=ibig[:, t0 * BL:(t0 + TC) * BL],
                                                in0=hc[:], in1=mrep[:], op=OP.mult)

            # ---- phase 4: final MLP ----
            with (
                tc.tile_pool(name="p4s", bufs=1) as p4,
                tc.tile_pool(name="p4p", bufs=1, space="PSUM") as q4,
            ):
                ilast = ibig[:, (ss - 1) * BL:ss * BL]
                rhs_list = [userT[:], itemT[:], catT[:DC, :], hau[:], ilast, featT_s[:]]
                pf1a = q4.tile([128, BL], F32, name="pf1a")
                pf1b = q4.tile([128, BL], F32, name="pf1b")
                for mi, (pf, mlo) in enumerate(((pf1a, 0), (pf1b, 128))):
                    for j in range(6):
                        nc.tensor.matmul(out=pf[:], lhsT=fc1k_s[j][:, mlo:mlo + 128],
                                         rhs=rhs_list[j], start=(j == 0), stop=(j == 5))
                x1a = p4.tile([128, BL], BF16, name="x1a")
                x1b = p4.tile([128, BL], BF16, name="x1b")
                nc.scalar.activation(out=x1a[:], in_=pf1a[:], func=AF.Relu, bias=fb1a_s[:, :1])
                nc.scalar.activation(out=x1b[:], in_=pf1b[:], func=AF.Relu, bias=fb1b_s[:, :1])
                pf2 = q4.tile([128, BL], F32, name="pf2")
                nc.tensor.matmul(out=pf2[:], lhsT=fc2a_s[:], rhs=x1a[:], start=True, stop=False)
                nc.tensor.matmul(out=pf2[:], lhsT=fc2b_s[:], rhs=x1b[:], start=False, stop=True)
                x2 = p4.tile([128, BL], BF16, name="x2")
                nc.scalar.activation(out=x2[:], in_=pf2[:], func=AF.Relu, bias=fb2_s[:, :1])
                pf3 = q4.tile([64, BL], F32, name="pf3")
                nc.tensor.matmul(out=pf3[:], lhsT=fc3_s[:], rhs=x2[:], start=True, stop=True)
                x3 = p4.tile([64, BL], BF16, name="x3")
                nc.scalar.activation(out=x3[:], in_=pf3[:], func=AF.Relu, bias=fb3_s[:, :1])
                pf4 = q4.tile([1, BL], F32, name="pf4")
                nc.tensor.matmul(out=pf4[:], lhsT=fc4_s[:], rhs=x3[:], start=True, stop=True)
                y = p4.tile([1, BL], F32, name="y")
                nc.scalar.activation(out=y[:], in_=pf4[:], func=AF.Sigmoid, bias=fb4_s[:1, :1])
                nc.sync.dma_start(out=out[:], in_=y[:])

    nc.compile()
    return nc


def get_module(ss=S):
    if ss not in _BUILT:
        _BUILT[ss] = _build(ss)
    return _BUILT[ss]


def host_prep(inputs, ss=S):
    """Build the 8 per-core input maps from full inputs."""
    f32 = np.float32
    gi = {k: np.asarray(v) for k, v in inputs.items()}
    gru_Wih, gru_Whh = gi["gru_Wih"].astype(f32), gi["gru_Whh"].astype(f32)
    gru_bih, gru_bhh = gi["gru_bih"].astype(f32), gi["gru_bhh"].astype(f32)

    ctab = np.zeros((NC, 128), f32)
    ctab[:, :DC] = gi["cat_table"].astype(f32)
    ctab[:, DC] = 1.0       # mask flag: 1 = valid step (must be @64 for MM)
    ctab[:, DC + 1] = 1.0   # bias-carrier column
    # pad rows appended to tabic: zero item row; cat-pad row with z-saturate
    # flag set and mask flag 0
    pad_item = np.zeros((1, 128), f32)
    pad_cat = np.zeros((1, 128), f32)
    pad_cat[0, DC + 1] = 1.0
    pad_cat[0, DC + 2] = 1.0
    bias_row = gru_bih + np.concatenate([gru_bhh[:H], gru_bhh[H:2 * H], np.zeros(H, f32)])
    wih_c = np.zeros((128, 3 * H), f32)
    wih_c[:DC] = gru_Wih[:, D:].T
    wih_c[DC + 1] = bias_row
    wih_c[DC + 2, H:2 * H] = 40.0  # pad steps: z pre-act += 40 -> sigmoid == 1

    att_W1 = gi["att_W1"].astype(f32)

    def bf(x):
        return np.ascontiguousarray(np.asarray(x, f32).astype(np_bf16))

    fc1_W = gi["fc1_W"].astype(f32)
    bounds = np.cumsum([0] + KCH_HOST)
    fc2_W = gi["fc2_W"].astype(f32)
    onesb = np.zeros((128, BL), f32)
    onesb[DC] = 1.0

    wblobs = {
        "wih_i": gru_Wih[:, :D].T, "wih_c": wih_c, "whh": gru_Whh.T,
        "w1i": att_W1[:, :H].T, "w1ti": att_W1[:, H:H + D].T,
        "w1tc": att_W1[:, H + D:].T,
        "w2": gi["att_W2"].astype(f32).T,
        "w3r": np.tile(gi["att_W3"].astype(f32).T, (1, 128)),
        "wir": gi["au_Wir"].astype(f32).T, "wiha": gi["au_Wih"].astype(f32).T,
        "whr": gi["au_Whr"].astype(f32).T, "whha": gi["au_Whh"].astype(f32).T,
        "fc2a": fc2_W[:, :128].T, "fc2b": fc2_W[:, 128:].T,
        "fc3": gi["fc3_W"].astype(f32).T, "fc4": gi["fc4_W"].astype(f32).T,
        "onesb": onesb, "id80": np.eye(80, dtype=f32),
    }
    for j in range(6):
        wblobs[f"fc1k{j}"] = fc1_W[:, bounds[j]:bounds[j + 1]].T
    fblobs = {
        "bhhn": gru_bhh[2 * H:].reshape(H, 1),
        "b1": gi["att_b1"].reshape(80, 1), "b2": gi["att_b2"].reshape(40, 1),
        "b3r": np.full((128, 1), gi["att_b3"][0], f32),
        "br": gi["au_br"].reshape(H, 1), "bh": gi["au_bh"].reshape(H, 1),
        "fb1a": gi["fc1_b"][:128].reshape(128, 1),
        "fb1b": gi["fc1_b"][128:].reshape(128, 1),
        "fb2": gi["fc2_b"].reshape(128, 1), "fb3": gi["fc3_b"].reshape(64, 1),
        "fb4": gi["fc4_b"].reshape(1, 1),
    }
    wpk0 = np.zeros((128, WBF_N), np_bf16)
    for name, arr in wblobs.items():
        o, r, cs = WBF_OFF[name]
        wpk0[:r, o:o + cs] = np.asarray(arr, f32).astype(np_bf16)
    fpk0 = np.zeros((128, WF32_N), f32)
    for name, arr in fblobs.items():
        o, r, _ = WF32_OFF[name]
        fpk0[:r, o:o + 1] = np.asarray(arr, f32)

    shared = dict(
        utab=bf(gi["user_table"]),
        tabic=np.concatenate(
            [bf(gi["item_table"]), bf(ctab), bf(pad_item), bf(pad_cat)], 0),
        fpk=fpk0,
    )

    lens = np.maximum(gi["seq_lens"].astype(np.int64), 1)
    mask_full = (np.arange(ss)[:, None] < lens[None, :]).astype(f32)  # [ss, B]

    fo, fr, fcs = WBF_OFF["featT"]
    in_maps = []
    for c in range(NCORES):
        bs = slice(c * BL, (c + 1) * BL)
        mB = mask_full[:, bs].T.astype(bool)        # [BL, ss]
        hx = np.empty((BL, 2 * ss), np.int32)
        hx[:, 0::2] = np.where(mB, gi["hist_items"][bs, :ss], NI + NC)
        hx[:, 1::2] = np.where(mB, NI + gi["hist_cats"][bs, :ss], NI + NC + 1)
        wpkc = wpk0.copy()
        wpkc[:fr, fo:fo + fcs] = gi["features"][bs].astype(f32).astype(np_bf16).T
        im = dict(shared)
        im.update(
            ids3=np.ascontiguousarray(np.stack(
                [gi["user_ids"][bs], gi["article_ids"][bs],
                 NI + gi["category_ids"][bs]], 1), np.int32),
            hidx=hx,
            wpk=wpkc,
        )
        in_maps.append(im)
    return in_maps


def kernel(**inputs):
    nc = get_module(S)
    in_maps = host_prep(inputs, S)
    res = run_bass_kernel_spmd(nc, in_maps, core_ids=list(range(NCORES)))
    outs = [res.results[c]["out"].reshape(BL, 1) for c in range(NCORES)]
    return np.concatenate(outs, 0).astype(np.float32)

